# revision 28
# baseline (speedup 1.0000x reference)
"""Trainium2 Bass kernel for Ernie4.5 attention (B=1, S=2048, HID=4096, H=32,
KVH=8, D=128), tensor-parallel over heads across 8 NeuronCores.

Core i owns q-heads 4i..4i+3, kv-head i, and wo rows [512*i, 512*(i+1)).
Each core computes its partial output [S, HID] (fp16); the host sums the 8
partials in f32.

Per-core pipeline (transposed [feature, seq] activation layouts throughout),
emitted so attention chunk j interleaves with projection chunk j+1 (the tile
scheduler fills PE stalls of one phase with work from the other):
  1. qT/kT/vT = (w.T @ hsT-tiles), weights stationary      -> [D, S] tiles
  2. RoPE on qT/kT via stream_shuffle + fp16 host tables
  3. per head: scoresT[sk,sq] pairs of 128-blocks land in 2-bank PSUM tiles;
     one exp per pair (bias=-3 so fp16 row-sum accumulation is range-safe);
     causal diag masking via 0/1 pair masks; softmax denominators accumulated
     on the Vector engine in fp16 (not on the PE) and all-reduced across
     partitions on GpSimd; outT[d,sq] += v.T @ probsT in PSUM; normalize via
     reciprocal_approx_fast + vector mul
  4. final[sq,hid] = sum_c outT[c].T @ wo[c] -> fp16 partial, written as
     [128,1024] column-pair tiles with one DMA each (dispatch alternates
     between the Sync and GpSimd queues)
"""

import os
import sys
from contextlib import ExitStack

import numpy as np

for _p in ("/opt/trn_rl_repo",):
    if os.path.isdir(_p) and _p not in sys.path:
        sys.path.append(_p)

import ml_dtypes

import concourse.bass as bass
import concourse.bass_isa as bass_isa
import concourse.mybir as mybir
import concourse.tile as tile
from concourse import bacc
from concourse.bass_utils import run_bass_kernel_spmd
from concourse.masks import make_identity

P = 128
B, S, HID, H, KVH, D = 1, 2048, 4096, 32, 8, 128
NCORES = 8
HL = H // NCORES          # 4 local q heads
NKT = HID // P            # 32 contraction tiles
NSQ = S // P              # 16 seq blocks
CW = 512                  # seq chunk width
NCH = S // CW             # 4 seq chunks
KP = 4                    # hsT k-tiles packed per DMA
PBUFS = 10                # hsT pack ring: one chunk (8) + slack
WOC = 512                 # wo output chunk width
NHC = HID // WOC          # 8 wo output chunks
NCB = HL + 2              # 6 projection column blocks (4 q heads, k, v)
SCALE = float(D) ** -0.5
EXPB = -3.0               # exp bias: cancels in softmax, keeps fp16 sums small
BASE = 10000.0

F32 = mybir.dt.float32
BF16 = mybir.dt.bfloat16
FP16 = mybir.dt.float16
SWAP_MASK = [i ^ 1 for i in range(32)]

LAST_RESULT = None


def _build(act_dt=BF16):
    # teach the tile scheduler's cost model the measured GpSimd op rates so
    # it schedules dependents of the slow partition ops far enough downstream
    from concourse.hw_specs import TRN2Spec
    TRN2Spec.GPSIMD_IMPL_EFFICIENCY = {
        **TRN2Spec.GPSIMD_IMPL_EFFICIENCY,
        "PartitionAllReduce": 0.118,
        "PartitionBroadcast": 0.36,
    }
    nc = bacc.Bacc("TRN2", target_bir_lowering=False, debug=False)

    hsT_d = nc.dram_tensor("hsT", [HID, S], act_dt, kind="ExternalInput").ap()
    wqkv_d = nc.dram_tensor("wqkv", [NCB, P, NKT * P], act_dt, kind="ExternalInput").ap()
    wo_d = nc.dram_tensor("wo", [HL, P, NHC, WOC], act_dt, kind="ExternalInput").ap()
    cosT_d = nc.dram_tensor("cosT", [P, S], FP16, kind="ExternalInput").ap()
    ssinT_d = nc.dram_tensor("ssinT", [P, S], FP16, kind="ExternalInput").ap()
    dmask_d = nc.dram_tensor("dmask", [P, 2, 2 * CW], FP16, kind="ExternalInput").ap()
    out_d = nc.dram_tensor("out", [S, HID], FP16, kind="ExternalOutput").ap()

    with tile.TileContext(nc) as tc, ExitStack() as ctx:
        const = ctx.enter_context(tc.tile_pool(name="const", bufs=1))
        wpool = ctx.enter_context(tc.tile_pool(name="wpool", bufs=1))
        tabs = ctx.enter_context(tc.tile_pool(name="tabs", bufs=1))
        res = ctx.enter_context(tc.tile_pool(name="res", bufs=1))
        hst = ctx.enter_context(tc.tile_pool(name="hst", bufs=PBUFS))
        evq = ctx.enter_context(tc.tile_pool(name="evq", bufs=3))
        rope = ctx.enter_context(tc.tile_pool(name="rope", bufs=2))
        vtmp = ctx.enter_context(tc.tile_pool(name="vtmp", bufs=2))
        probs = ctx.enter_context(tc.tile_pool(name="probs", bufs=6))
        dacc = ctx.enter_context(tc.tile_pool(name="dacc", bufs=3))
        posb = ctx.enter_context(tc.tile_pool(name="posb", bufs=3))
        norm = ctx.enter_context(tc.tile_pool(name="norm", bufs=2))
        wow = ctx.enter_context(tc.tile_pool(name="wow", bufs=12))
        outsb = ctx.enter_context(tc.tile_pool(name="outsb", bufs=5))
        # PSUM: 8 banks. psP = 2x two-bank [128,1024] tiles (score pairs;
        # halves double as single-bank accumulators in phases 1/3),
        # psO = 2x one-bank, psM = 2x one-bank (v transposes).
        psP = ctx.enter_context(tc.tile_pool(name="psP", bufs=2, space="PSUM"))
        psO = ctx.enter_context(tc.tile_pool(name="psO", bufs=3, space="PSUM"))
        psM = ctx.enter_context(tc.tile_pool(name="psM", bufs=1, space="PSUM"))

        # ---- input DMA: interleave weight k-slabs with chunk-0 hsT packs so
        # the first projection matmuls can start as soon as slices land.
        w_all = wpool.tile([P, NCB, NKT * P], act_dt)
        _hsT_r = hsT_d.rearrange("(g kp p) s -> g p kp s", g=NKT // KP, kp=KP, p=P)
        hst_tiles = {}

        def _load_hst_pack(p, g, split=False):
            t = hst.tile([P, KP, CW], act_dt, tag="hst", name=f"hst_{p}_{g}")
            if split:  # chunk 0: per-k-slice DMAs so the first arrives fast
                for kp in range(KP):
                    nc.sync.dma_start(t[:, kp, :],
                                      _hsT_r[g, :, kp, bass.ds(p * CW, CW)])
            else:
                nc.sync.dma_start(t[:], _hsT_r[g, :, :, bass.ds(p * CW, CW)])
            hst_tiles.setdefault(p, {})[g] = t

        def _load_w(c, g, eng=None):
            (eng or nc.sync).dma_start(
                w_all[:, c, g * KP * P:(g + 1) * KP * P],
                wqkv_d[c, :, g * KP * P:(g + 1) * KP * P])

        def _load_hst_kp(g, kp, eng=None):
            key = ('t', g)
            if key not in hst_tiles.setdefault(0, {}):
                hst_tiles[0][key] = hst.tile([P, KP, CW], act_dt, tag="hst",
                                             name=f"hst_0_{g}")
            t = hst_tiles[0][key]
            (eng or nc.sync).dma_start(t[:, kp, :],
                                       _hsT_r[g, :, kp, bass.ds(0, CW)])
            hst_tiles[0][g] = t

        for c, job in ((0, None), (None, (0, 0)), (None, (0, 1)), (1, None),
                       (None, (0, 2)), (2, None), (None, (0, 3)), (3, None),
                       (4, None), (5, None)):
            if c is not None:
                _load_w(c, 0)
            else:
                _load_hst_kp(*job)
        for g in range(1, NKT // KP):
            for c in range(NCB):
                _load_w(c, g)
                if c == 0:
                    _load_hst_pack(0, g)

        ones16 = const.tile([P, 1], FP16)
        nc.vector.memset(ones16[:], 1.0)
        nbias = const.tile([P, 1], F32)
        nc.vector.memset(nbias[:], EXPB)

        cosT = tabs.tile([P, S], FP16)
        nc.sync.dma_start(cosT[:], cosT_d[:, :])
        ssinT = tabs.tile([P, S], FP16)
        nc.sync.dma_start(ssinT[:], ssinT_d[:, :])
        dmask = tabs.tile([P, 2, 2 * CW], FP16)
        nc.sync.dma_start(dmask[:], dmask_d[:, :, :])

        # resident activations
        qkT = res.tile([P, HL + 1, S], act_dt)
        v_sb = res.tile([P, NSQ, P], FP16)
        outT = res.tile([P, HL, S], act_dt)

        # ---- phase 1 helpers: projections + RoPE + (deferred) v transposes
        def _psum6(pfx):
            pa = psP.tile([P, 2 * CW], F32, tag="p", name=f"{pfx}_a")
            pb_ = psP.tile([P, 2 * CW], F32, tag="p", name=f"{pfx}_b")
            o1 = psO.tile([P, CW], F32, tag="o", name=f"{pfx}_o1")
            o2 = psO.tile([P, CW], F32, tag="o", name=f"{pfx}_o2")
            return [pa[:, 0:CW], pa[:, CW:2 * CW], pb_[:, 0:CW],
                    pb_[:, CW:2 * CW], o1[:], o2[:]]

        vstash = {}

        def _proj_chunk(p):
            packs = hst_tiles.pop(p)
            ps6 = _psum6(f"pj{p}")

            def mm(c, k):
                nc.tensor.matmul(
                    ps6[c], w_all[:, c, k * P:(k + 1) * P],
                    packs[k // KP][:, k % KP, :],
                    start=(k == 0), stop=(k == NKT - 1))

            if p == 0:
                # start in DMA-arrival order, then c-outer so the six
                # accumulator stops (and evictions) stagger
                order = [(0, 0), (0, 1), (1, 0), (0, 2), (2, 0), (0, 3),
                         (3, 0), (4, 0), (5, 0)]
                order += [(c, k) for k in (1, 2, 3) for c in (1, 2, 3, 4, 5)]
                for c, k in order:
                    mm(c, k)
                for c in range(NCB):
                    for k in range(KP, NKT):
                        mm(c, k)
            else:
                for c in range(NCB):
                    for k in range(NKT):
                        mm(c, k)
            return ps6

        def _finish_block(p, c, ps):
            osl = bass.ds(p * CW, CW)
            if c < HL + 1:  # q heads and k: RoPE then store
                raw = evq.tile([P, CW], act_dt, tag="raw")
                if c % 2 == 0:
                    nc.scalar.copy(raw[:], ps)
                else:
                    nc.vector.tensor_copy(raw[:], ps)
                t1 = rope.tile([P, CW], act_dt, tag="t1")
                nc.vector.tensor_mul(t1[:], raw[:], cosT[:, osl])
                t2 = rope.tile([P, CW], act_dt, tag="t2")
                nc.vector.stream_shuffle(t2[:], raw[:], SWAP_MASK)
                t3 = rope.tile([P, CW], act_dt, tag="t3")
                nc.vector.tensor_mul(t3[:], t2[:], ssinT[:, osl])
                nc.vector.tensor_add(qkT[:, c, osl], t1[:], t3[:])
            else:  # v: evict, transpose later at a PE slack point
                vt = vtmp.tile([P, CW], FP16, tag="vt")
                nc.scalar.copy(vt[:], ps)
                vstash[p] = vt

        def _drain_v(p):
            vt = vstash.pop(p)
            for b in range(CW // P):
                nc.sync.dma_start_transpose(v_sb[:, p * (CW // P) + b, :],
                                            vt[:, b * P:(b + 1) * P])

        # ---- attention for one (head, chunk): scoresT pairs, fp16 rowsums
        def _attention(h, j):
            jsl = bass.ts(j, CW)
            nblk = (j + 1) * (CW // P)
            po = psO.tile([P, CW], F32, tag="o", name=f"po_{h}_{j}")
            acc2 = dacc.tile([P, 2 * CW], FP16, tag="da", name=f"da_{h}_{j}")
            for pi in range(nblk // 2):
                sk0 = 2 * pi
                t0 = sk0 - j * (CW // P)   # >= 0 iff diagonal pair
                op = t0 * P if t0 > 0 else 0
                ppair = psP.tile([P, 2 * CW], F32, tag="p", name=f"sc_{h}_{j}_{pi}")
                for half in (0, 1):
                    sk = sk0 + half
                    t = sk - j * (CW // P)
                    o = t * P if t > 0 else 0
                    csl = bass.ds(j * CW + o, CW - o)
                    nc.tensor.matmul(
                        ppair[:, half * CW + o:(half + 1) * CW],
                        qkT[:, HL, sk * P:(sk + 1) * P], qkT[:, h, csl],
                        start=True, stop=True)
                pb = probs.tile([P, 2 * CW], FP16, tag="pb")
                nc.scalar.activation(
                    pb[:, op:], ppair[:, op:],
                    mybir.ActivationFunctionType.Exp,
                    bias=nbias[:], scale=SCALE)
                if t0 >= 0:  # both halves masked (incl. exp'd junk cols)
                    nc.vector.tensor_mul(pb[:, op:], pb[:, op:],
                                         dmask[:, t0 // 2, op:])
                for half in (0, 1):
                    sk = sk0 + half
                    t = sk - j * (CW // P)
                    o = t * P if t > 0 else 0
                    mv = pb[:, half * CW + o:(half + 1) * CW]
                    nc.tensor.matmul(po[:, o:], v_sb[:, sk, :], mv,
                                     start=(sk == 0), stop=(sk == nblk - 1))
                # denominator accumulation: one pair-wide DVE op (masked
                # zeros make the trimmed-but-covered columns harmless)
                if pi == 0:
                    nc.vector.tensor_copy(acc2[:], pb[:])
                else:
                    nc.vector.tensor_add(acc2[:, op:], acc2[:, op:], pb[:, op:])
            # evict po to SBUF right away so the PSUM bank frees regardless
            # of how long the normalize chain takes
            poS = posb.tile([P, CW], FP16, tag="po", name=f"poS_{h}_{j}")
            if h % 2 == 0:
                nc.vector.tensor_copy(poS[:], po[:])
            else:
                nc.scalar.copy(poS[:], po[:])
            # denominator: fold acc2 halves on DVE, then one cross-partition
            # ones-matmul, then recip + broadcast + scale
            accf = dacc.tile([P, CW], FP16, tag="df", name=f"df_{h}_{j}")
            nc.vector.tensor_add(accf[:], acc2[:, 0:CW], acc2[:, CW:2 * CW])
            pr = psM.tile([1, CW], F32, tag="m", name=f"pr_{h}_{j}")
            nc.tensor.matmul(pr[:], ones16[:], accf[:], start=True, stop=True)
            rc = norm.tile([1, CW], F32, tag="rc", name=f"rc_{h}_{j}")
            nc.vector.reciprocal_approx_fast(rc[:], pr[:])
            rb = norm.tile([P, CW], F32, tag="rb", name=f"rb_{h}_{j}")
            nc.gpsimd.partition_broadcast(rb[:], rc[:], channels=P)
            nc.vector.tensor_mul(outT[:, h, jsl], poS[:], rb[:])

        # ---- wo weight prefetch (pairs of output chunks)
        def _load_wo_pair(hcp):
            wts = []
            for hc in (2 * hcp, 2 * hcp + 1):
                for c in range(HL):
                    wt = wow.tile([P, WOC], act_dt, tag="wt")
                    nc.gpsimd.dma_start(wt[:], wo_d[c, :, hc, :])
                    wts.append(wt)
            return wts

        # ---- emission: chunk p projections interleave with attention j=p-1
        for p in range(NCH):
            if p + 1 < NCH:
                for g in range(NKT // KP):
                    _load_hst_pack(p + 1, g)
            ps6 = _proj_chunk(p)
            if p >= 1:
                _drain_v(p - 1)
            for c in range(NCB):
                _finish_block(p, c, ps6[c])
            if p == 1:
                wts_cur = _load_wo_pair(0)
            if p == NCH - 1:
                _drain_v(NCH - 1)
            if p >= 1:
                for h in range(HL):
                    _attention(h, p - 1)
        for h in range(HL):
            _attention(h, NCH - 1)

        # ---- phase 3: wo projection over column pairs
        pf_box = [None]

        def _alloc_pfpair(i, name):
            r = i % 3
            if r < 2:
                pp = psP.tile([P, 2 * CW], F32, tag="p", name=name)
                return pp[:, 0:CW], pp[:, CW:2 * CW], pp[:], True
            o1 = psO.tile([P, CW], F32, tag="o", name=f"{name}a")
            o2 = psO.tile([P, CW], F32, tag="o", name=f"{name}b")
            return o1[:], o2[:], None, False

        for hcp in range(NHC // 2):
            wts = wts_cur
            wts_cur = _load_wo_pair(hcp + 1) if hcp + 1 < NHC // 2 else None
            ocols = bass.ds(hcp * 2 * WOC, 2 * WOC)
            for sq in range(NSQ):
                i = hcp * NSQ + sq
                pf0, pf1, pfull, fused = _alloc_pfpair(i, f"pf_{hcp}_{sq}")
                for c in range(HL):
                    nc.tensor.matmul(pf0, outT[:, c, sq * P:(sq + 1) * P],
                                     wts[c][:], start=(c == 0), stop=(c == HL - 1))
                for c in range(HL):
                    nc.tensor.matmul(pf1, outT[:, c, sq * P:(sq + 1) * P],
                                     wts[HL + c][:], start=(c == 0), stop=(c == HL - 1))
                ob = outsb.tile([P, 2 * WOC], FP16, tag="ob")
                eng = nc.vector.tensor_copy if sq % 2 == 0 else (
                    lambda o, i_: nc.scalar.copy(o, i_))
                if fused:
                    eng(ob[:], pfull)
                else:
                    eng(ob[:, 0:WOC], pf0)
                    eng(ob[:, WOC:2 * WOC], pf1)
                dma_eng = nc.gpsimd if i % 2 == 0 else nc.sync
                orows = bass.ds(sq * P, P)
                if hcp == NHC // 2 - 1:
                    # final column pair: smaller parallel pieces so the
                    # drain after the last matmul stays short
                    nsp = 4 if sq >= NSQ - 4 else 2
                    for hp in range(nsp):
                        rsl = bass.ds(sq * P + hp * (P // nsp), P // nsp)
                        dma_eng.dma_start(out_d[rsl, ocols],
                                          ob[bass.ds(hp * (P // nsp), P // nsp), :])
                        dma_eng = nc.sync if dma_eng is nc.gpsimd else nc.gpsimd
                else:
                    dma_eng.dma_start(out_d[orows, ocols], ob[:])

    nc.compile()
    return nc


def _rope_tables():
    inv_freq = (1.0 / (BASE ** (np.arange(0, D, 2, dtype=np.float32) / D))).astype(np.float32)
    pos = np.arange(S, dtype=np.float32)[:, None]
    ang = pos * inv_freq[None, :]
    sin = np.sin(ang).astype(np.float32).T     # [D/2, S]
    cos = np.cos(ang).astype(np.float32).T
    cosT = np.empty((D, S), np.float32)
    cosT[0::2] = cos
    cosT[1::2] = cos
    ssinT = np.empty((D, S), np.float32)
    ssinT[0::2] = -sin
    ssinT[1::2] = sin
    return cosT.astype(np.float16), ssinT.astype(np.float16)


def _diag_masks():
    # pair masks: dmask[p, i, F] = 1 where scoreT element (sk=128*(2i+h)+p,
    # sq=F-512h) with h=F//512 is causal-valid; 0 elsewhere (incl. junk cols)
    p = np.arange(P)[:, None, None]
    i = np.arange(2)[None, :, None]
    F = np.arange(2 * CW)[None, None, :]
    h = F // CW
    f = F - CW * h
    return (f >= P * (2 * i + h) + p).astype(np.float16)


_NC_CACHE = {}


def kernel(hidden_states, wq, wk, wv, wo):
    global LAST_RESULT
    act_np = ml_dtypes.bfloat16
    key = "v3"
    if key not in _NC_CACHE:
        _NC_CACHE[key] = _build()
    nc = _NC_CACHE[key]

    hs = np.asarray(hidden_states, np.float32).reshape(S, HID)
    hsT = np.ascontiguousarray(hs.T).astype(act_np)
    cosT, ssinT = _rope_tables()
    dmask = _diag_masks()

    in_maps = []
    for i in range(NCORES):
        wqkv = np.concatenate(
            [np.asarray(wq, np.float32)[:, i * HL * D:(i + 1) * HL * D],
             np.asarray(wk, np.float32)[:, i * D:(i + 1) * D],
             np.asarray(wv, np.float32)[:, i * D:(i + 1) * D]], axis=1)
        # [HID, 768] -> [NCB, P, NKT*P]: block c, hid-in-tile p, (k-tile, col)
        wqkv = np.ascontiguousarray(
            wqkv.reshape(NKT, P, NCB, P).transpose(2, 1, 0, 3).reshape(NCB, P, NKT * P)
        ).astype(act_np)
        wo_i = np.ascontiguousarray(
            np.asarray(wo, np.float32)[i * HL * D:(i + 1) * HL * D, :]
            .reshape(HL, P, NHC, WOC)).astype(act_np)
        in_maps.append({
            "hsT": hsT, "wqkv": wqkv, "wo": wo_i,
            "cosT": cosT, "ssinT": ssinT, "dmask": dmask,
        })

    trace = bool(os.environ.get("BASS_KERNEL_TRACE"))
    res = run_bass_kernel_spmd(nc, in_maps, list(range(NCORES)),
                               trace=trace, trace_cores=[0] if trace else None)
    LAST_RESULT = res
    acc = np.zeros((S, HID), np.float32)
    for i in range(NCORES):
        acc += np.asarray(res.results[i]["out"], np.float32)
    return acc.reshape(B, S, HID)
